# revision 37
# baseline (speedup 1.0000x reference)
"""Causal multi-head attention (B=2, S=2048, D=1024, H=16, Dh=64) on 8 TRN2 cores.

Sharding: core c -> batch b=c//4, head-group g=c%4 (heads 4g..4g+3, d_out cols
g*256..(g+1)*256). Each core computes Q/K/V projections for its head group from
x[b] and runs causal attention for its 4 heads independently. No collectives.

All matmuls run in fp16 (1 cyc/col warm). Precision budget (rel err < 2e-2):
  proj: q = x*wq_hi in 2 passes (xhi*wq + xlo*wq); k,v 1-pass (xhi*w).
  scores: 3 terms, all as K=64 row-tiled PAIRS (heads 2j/2j+1 concurrent on
  PE row groups (0,0)/(64,0)):  s = q_hi*k_hi + q_lo*k_hi + q_hi*k_lo
  where q_hi/q_lo (k_hi/k_lo) are exact fp16 hi/lo splits of the proj psum.
  Predicted rel err ~1.1e-2 (numpy emulation of the exact pipeline).

Softmax: a cheap paired "minimax" pre-pass (q_hi*k_hi only, masked, chunk
reduce_max) computes -max per q row BEFORE the main score pass, so the main
pass exp (ACT, bias=-max) fires per-chunk as soon as its psum accumulation
stops -> psum banks recycle fast and exp never serializes behind a global
reduce. p rows (fp16) are transposed via the DMA xbar into [k-part, q] tiles
for PV. V carries a ones column so PV also yields the softmax denominator;
host divides and assembles.
"""

import math

import numpy as np

B = 2
SEQ = 2048
DIN = 1024
H = 16
DH = 64
NCORES = 8
DO = 256  # d_out columns per core (4 heads)
HPC = 4  # heads per core
KT_N = DIN // 128  # 8 contraction tiles
ST_N = SEQ // 128  # 16 seq tiles
QC_N = SEQ // 512  # 4 q-chunks
NEG = -30000.0
Q_PASS = 2  # 1 or 2 passes for the q projection

_CACHE = {}
LAST_RESULTS = None


def _emit_core_kernel(tc, outs, ins):
    from concourse import mybir

    nc = tc.nc
    f32 = mybir.dt.float32
    f16 = mybir.dt.float16
    (outT,) = outs  # [HPC, 65, SEQ] f32
    xhi, xlo, wq, wk, wv, mask = ins

    from contextlib import ExitStack

    AX = mybir.AxisListType.X

    with ExitStack() as ctx:
        consts = ctx.enter_context(tc.tile_pool(name="consts", bufs=1))
        xs_pool = ctx.enter_context(tc.tile_pool(name="xs", bufs=2))
        qk_pool = ctx.enter_context(tc.tile_pool(name="qk", bufs=1))
        v_pool = ctx.enter_context(tc.tile_pool(name="vp", bufs=1))
        prow_pool = ctx.enter_context(tc.tile_pool(name="prow", bufs=3))
        pt_pool = ctx.enter_context(tc.tile_pool(name="pt", bufs=2))
        stats = ctx.enter_context(tc.tile_pool(name="stats", bufs=3))
        outp = ctx.enter_context(tc.tile_pool(name="outp", bufs=2))
        ps_sc = ctx.enter_context(
            tc.tile_pool(name="ps_sc", bufs=5, space="PSUM")
        )
        ps_proj = ctx.enter_context(
            tc.tile_pool(name="ps_proj", bufs=2, space="PSUM")
        )
        ps_o = ctx.enter_context(tc.tile_pool(name="ps_o", bufs=1, space="PSUM"))

        mask_sb = consts.tile([128, 128], f32, tag="mask", name="mask")
        nc.sync.dma_start(mask_sb[:], mask[:])
        w_sb = {}
        for wname, wap in (("wk", wk), ("wq", wq), ("wv", wv)):
            t = consts.tile([128, KT_N, DO], f16, tag=wname, name=f"{wname}_sb")
            nc.sync.dma_start(t[:], wap.rearrange("(k p) n -> p k n", p=128))
            w_sb[wname] = t

        # per head-pair j: heads 2j (parts 0-63) and 2j+1 (parts 64-127),
        # columns = seq positions. hi/lo are exact fp16 splits of proj psum.
        q_hi = [qk_pool.tile([128, SEQ], f16, tag=f"qh{j}", name=f"qh{j}") for j in range(2)]
        q_lo = [qk_pool.tile([128, SEQ], f16, tag=f"ql{j}", name=f"ql{j}") for j in range(2)]
        k_hi = [qk_pool.tile([128, SEQ], f16, tag=f"kh{j}", name=f"kh{j}") for j in range(2)]
        k_lo = [qk_pool.tile([128, SEQ], f16, tag=f"kl{j}", name=f"kl{j}") for j in range(2)]
        v_sb = [
            v_pool.tile([128, HPC, DH + 1], f16, tag=f"v{s}", name=f"v{s}")
            for s in range(ST_N)
        ]

        def load_x(sc):
            # single batched DMA per tensor: 2 gpsimd issues/phase, not 16
            # (the Q7 descriptor-gen is ~650ns per dma_start and head-blocks
            # the affine_selects/out-stores behind it)
            ssl = slice(sc * 512, (sc + 1) * 512)
            th = xs_pool.tile([128, KT_N, 512], f16, tag="xhi", name="xhi_sb")
            nc.gpsimd.dma_start(
                th[:], xhi[:, ssl].rearrange("(k p) s -> p k s", p=128)
            )
            xh = [th[:, k, :] for k in range(KT_N)]
            xl = []
            if Q_PASS == 2:
                tl = xs_pool.tile([128, KT_N, 512], f16, tag="xlo", name="xlo_sb")
                nc.gpsimd.dma_start(
                    tl[:], xlo[:, ssl].rearrange("(k p) s -> p k s", p=128)
                )
                xl = [tl[:, k, :] for k in range(KT_N)]
            return xh, xl

        def proj_k(sc, xs, j):
            sl = slice(sc * 512, (sc + 1) * 512)
            xh, _ = xs
            msl = slice(j * 128, (j + 1) * 128)
            pst = ps_proj.tile([128, 512], f32, tag="pp", name="pk")
            for k in range(KT_N):
                nc.tensor.matmul(
                    pst[:], w_sb["wk"][:, k, msl], xh[k][:],
                    start=(k == 0), stop=(k == KT_N - 1),
                )
            nc.scalar.copy(k_hi[j][:, sl], pst[:])
            nc.vector.tensor_sub(k_lo[j][:, sl], pst[:], k_hi[j][:, sl])

        def proj_q(sc, xs, j):
            sl = slice(sc * 512, (sc + 1) * 512)
            xh, xl = xs
            msl = slice(j * 128, (j + 1) * 128)
            pst = ps_proj.tile([128, 512], f32, tag="pp", name="pq")
            n = KT_N * Q_PASS
            i = 0
            for k in range(KT_N):
                xaps = (xh[k],) if Q_PASS == 1 else (xh[k], xl[k])
                for xap in xaps:
                    nc.tensor.matmul(
                        pst[:], w_sb["wq"][:, k, msl], xap[:],
                        start=(i == 0), stop=(i == n - 1),
                    )
                    i += 1
            nc.scalar.copy(q_hi[j][:, sl], pst[:])
            nc.vector.tensor_sub(q_lo[j][:, sl], pst[:], q_hi[j][:, sl])

        def proj_v(sc, xs, jj):
            st = 4 * sc + jj
            xh, _ = xs
            psv = ps_proj.tile([128, 512], f32, tag="pp", name="pv")
            for k in range(KT_N):
                nc.tensor.matmul(
                    psv[:, 0:DO],
                    xh[k][:, jj * 128 : (jj + 1) * 128],
                    w_sb["wv"][:, k, :],
                    start=(k == 0),
                    stop=(k == KT_N - 1),
                )
            nc.scalar.copy(
                v_sb[st][:, :, 0:DH],
                psv[:, 0:DO].rearrange("p (h d) -> p h d", h=HPC),
            )
            nc.gpsimd.memset(v_sb[st][:, :, DH : DH + 1], 1.0)

        def proj_parts(sc, xs):
            return (
                [lambda j=j: proj_k(sc, xs, j) for j in range(2)]
                + [lambda j=j: proj_q(sc, xs, j) for j in range(2)]
                + [lambda jj=jj: proj_v(sc, xs, jj) for jj in range(4)]
            )

        mneg = {}  # (qt, h) -> AP [128,1] = -max of row scores

        def red_eng():
            # free-axis reduces are DVE-only (gpsimd reduces along C only)
            return nc.vector

        def minimax_unit(qt, j):
            """Paired q_hi*k_hi pass; masked diagonal; -max per row.
            Generator: yields after each chunk (for fine-grain interleave)."""
            L = (qt + 1) * 128
            qcols = slice(qt * 128, (qt + 1) * 128)
            nch = (L + 511) // 512
            mp = stats.tile([128, 2, 4], f32, tag=f"mp{qt % 2}{j}", name="mp")
            for ci in range(nch):
                c0 = ci * 512
                w = min(512, L - c0)
                psA = ps_sc.tile([128, 512], f32, tag="ps", name="mmA")
                psB = ps_sc.tile([128, 512], f32, tag="ps", name="mmB")
                nc.tensor.matmul(
                    psA[:, 0:w], q_hi[j][0:64, qcols], k_hi[j][0:64, c0 : c0 + w],
                    start=True, stop=True, tile_position=(0, 0),
                )
                nc.tensor.matmul(
                    psB[:, 0:w], q_hi[j][64:128, qcols],
                    k_hi[j][64:128, c0 : c0 + w],
                    start=True, stop=True, tile_position=(64, 0),
                )
                if c0 + w == L:
                    # gpsimd cannot touch PSUM; mask on DVE
                    nc.vector.tensor_add(
                        psA[:, w - 128 : w], psA[:, w - 128 : w], mask_sb[:]
                    )
                    nc.vector.tensor_add(
                        psB[:, w - 128 : w], psB[:, w - 128 : w], mask_sb[:]
                    )
                red_eng().reduce_max(
                    mp[:, 0, ci : ci + 1], psA[:, 0:w], axis=AX, negate=True
                )
                red_eng().reduce_max(
                    mp[:, 1, ci : ci + 1], psB[:, 0:w], axis=AX, negate=True
                )
                yield
            for hh in range(2):
                if nch > 1:
                    m = stats.tile(
                        [128, 1], f32, tag=f"mn{qt % 4}{j}{hh}", name="mn"
                    )
                    nc.vector.tensor_reduce(
                        m[:, 0:1], mp[:, hh, 0:nch], axis=AX,
                        op=mybir.AluOpType.min,
                    )
                    mneg[(qt, 2 * j + hh)] = m[:, 0:1]
                else:
                    mneg[(qt, 2 * j + hh)] = mp[:, hh, 0:1]

        _uidx = [0]

        def main_unit(qt, j, pt_tiles):
            """3-term paired scores + per-chunk exp + transpose.
            Generator: yields after each chunk-group."""
            L = (qt + 1) * 128
            qcols = slice(qt * 128, (qt + 1) * 128)
            nch = (L + 511) // 512
            pr = [
                prow_pool.tile([128, SEQ], f16, tag=f"pr{hh}{j}", name=f"pr{hh}{j}")
                for hh in range(2)
            ]
            qrows = (slice(0, 64), slice(64, 128))
            tpos = ((0, 0), (64, 0))
            # chunk groups of 2 -> at most 4 psum banks in flight per unit
            for g0 in range(0, nch, 2):
                cs = [
                    (ci, ci * 512, min(512, L - ci * 512))
                    for ci in range(g0, min(g0 + 2, nch))
                ]
                pst = {}
                for ci, c0, w in cs:
                    pst[ci] = (
                        ps_sc.tile([128, 512], f32, tag="ps", name="sA"),
                        ps_sc.tile([128, 512], f32, tag="ps", name="sB"),
                    )
                # term1: q_hi x k_hi (start) ; term3: q_hi x k_lo (same
                # stationary, no reload) ; term2: q_lo x k_hi (stop).
                # NOTE: dropping term3 measured SLOWER overall (352us vs
                # 292us): less PE work per unit -> sparser PE stream ->
                # HAM throttles to 1.2GHz. Keep it (also better precision).
                for term, qop, kop, st_, sp_ in (
                    (1, q_hi, k_hi, True, False),
                    (3, q_hi, k_lo, False, False),
                    (2, q_lo, k_hi, False, True),
                ):
                    for ci, c0, w in cs:
                        for hh in range(2):
                            nc.tensor.matmul(
                                pst[ci][hh][:, 0:w],
                                qop[j][qrows[hh], qcols],
                                kop[j][qrows[hh], c0 : c0 + w],
                                start=st_, stop=sp_,
                                tile_position=tpos[hh],
                                skip_group_check=True,
                            )
                for ci, c0, w in cs:
                    for hh in range(2):
                        ps_ = pst[ci][hh]
                        nc.scalar.activation(
                            pr[hh][:, c0 : c0 + w],
                            ps_[:, 0:w],
                            mybir.ActivationFunctionType.Exp,
                            bias=mneg[(qt, 2 * j + hh)],
                            scale=1.0,
                        )
                yield
            for hh in range(2):
                # sync-only: transpose issue occupies the engine ~1.5us each;
                # ACT must stay free for exp
                teng = nc.sync
                _uidx[0] += 1
                blk = slice((qt % 4) * 128, (qt % 4) * 128 + 128)
                teng.dma_start_transpose(
                    pt_tiles[2 * j + hh][:, : qt + 1, blk],
                    pr[hh][:, :L],
                )
                # diagonal block: exp'd junk above the diagonal (can be inf)
                # -> zero it AFTER the transpose (pt[k-part, q-free]: keep
                # q >= k) so the transpose doesn't wait on the gpsimd hop.
                nc.gpsimd.affine_select(
                    out=pt_tiles[2 * j + hh][:, qt, blk],
                    in_=pt_tiles[2 * j + hh][:, qt, blk],
                    compare_op=mybir.AluOpType.is_ge,
                    fill=0.0,
                    base=0,
                    pattern=[[1, 128]],
                    channel_multiplier=-1,
                )

        def pv_unit(qc, pt_tiles, h):
            po = ps_o.tile([65, 512], f32, tag="po", name="po")
            kt_hi = qc * 4 + 3
            for kt in range(kt_hi + 1):
                off = max(0, (kt - qc * 4)) * 128
                nc.tensor.matmul(
                    po[:, off:512],
                    v_sb[kt][:, h, :],
                    pt_tiles[h][:, kt, off:512],
                    start=(kt == 0),
                    stop=(kt == kt_hi),
                )
            ot = outp.tile([65, 512], f32, tag="ot", name="ot")
            nc.vector.tensor_copy(ot[:], po[:])
            nc.gpsimd.dma_start(outT[h, :, qc * 512 : (qc + 1) * 512], ot[:])

        def new_pt():
            return {
                h: pt_pool.tile(
                    [128, ST_N, 512], f16, tag=f"pt{h % 2}", name=f"pt{h % 2}"
                )
                for h in range(HPC)
            }

        # HAM warmup: the PE would otherwise idle ~5-8us waiting for the
        # first x/w DMAs and then run the whole first phase at K=4/8.
        # Burn ~24 dummy matmuls on a memset tile to keep activity up.
        warm_sb = consts.tile([128, 512], f16, tag="warm", name="warm")
        nc.gpsimd.memset(warm_sb[:], 0.0)
        for i in range(24):
            pw = ps_o.tile([65, 512], f32, tag="po", name="warmps")
            nc.tensor.matmul(
                pw[0:64, :], warm_sb[0:64, 0:64], warm_sb[0:64, :],
                start=True, stop=True,
            )

        # pipeline: proj(sc) ; PV(sc-1) ; scores(sc) with minimax LEAD=2
        # units ahead of main (DVE reduce burst of unit u overlaps the
        # reduce-free matmul stretch of main(u-2)).
        # sc=3 goes pair-major with PV(3) inline after each pair's mains.
        pt_by_qc = {}
        xs_cur = load_x(0)
        for sc in range(QC_N):
            xs_nxt = load_x(sc + 1) if sc < 3 else None
            if sc == 0:
                # proj(0) up front; proj(sc+1) for later phases is emitted
                # as tail-fill inside phase sc (below)
                for part in proj_parts(0, xs_cur):
                    part()
            xs_cur = xs_nxt
            # PV(sc-1) woven into the phase instead of head-blocking the PE
            # FIFO (it is transpose-gated; scores(sc) work behind it in the
            # queue is often ready first).
            pvgen = None
            if sc >= 1:
                def _pvfill(qc=sc - 1, pts=pt_by_qc[sc - 1]):
                    for h in range(HPC):
                        pv_unit(qc, pts, h)
                        yield
                pvgen = _pvfill()
            pt_by_qc[sc] = new_pt()
            if sc < 3:
                units = [(qt, j) for qt in range(sc * 4, sc * 4 + 4)
                         for j in range(2)]
            else:
                units = [(qt, j) for j in range(2) for qt in range(12, 16)]
            LEAD = 3
            # proj(sc+1) woven into this phase's tail (own psum pool, no
            # rotation contention; disjoint q/k column slices) so its
            # ACT/DVE copies are done before phase sc+1 starts.
            fillgen = None
            if sc < 3:
                def _fill(parts=proj_parts(sc + 1, xs_nxt)):
                    for part in parts:
                        part()
                        yield
                fillgen = _fill()
            fill_from = max(0, len(units) - 4)
            for i in range(len(units) + LEAD):
                # fine-grained interleave: one main chunk-group (6 MMs,
                # reduce-free) per minimax chunk (2 MMs + 2 slow DVE
                # reduces) keeps the PE fed while DVE digests.
                gens = []
                if i >= LEAD:
                    gens.append(main_unit(*units[i - LEAD], pt_by_qc[sc]))
                if i < len(units):
                    gens.append(minimax_unit(*units[i]))
                while gens:
                    for g in list(gens):
                        try:
                            next(g)
                        except StopIteration:
                            gens.remove(g)
                    if pvgen is not None and i >= 1:
                        try:
                            next(pvgen)
                        except StopIteration:
                            pvgen = None
                    if fillgen is not None and i >= fill_from:
                        try:
                            next(fillgen)
                        except StopIteration:
                            fillgen = None
                if sc == 3 and i >= LEAD and units[i - LEAD][0] == 15:
                    j = units[i - LEAD][1]
                    for hh in range(2):
                        pv_unit(3, pt_by_qc[sc], 2 * j + hh)
            while fillgen is not None:
                try:
                    next(fillgen)
                except StopIteration:
                    fillgen = None
            while pvgen is not None:
                try:
                    next(pvgen)
                except StopIteration:
                    pvgen = None
            if sc >= 1:
                pt_by_qc.pop(sc - 1)
            if sc < 3:
                # HAM bridge: keep the PE active across the phase handoff
                # (next phase's first units are ACT/DVE-copy-gated; an idle
                # PE here re-throttles to K=4/8 for ~10-17us)
                for _ in range(16):
                    pw = ps_o.tile([65, 512], f32, tag="po", name="bridge")
                    nc.tensor.matmul(
                        pw[0:64, :], warm_sb[0:64, 0:64], warm_sb[0:64, :],
                        start=True, stop=True,
                    )


def _split_waits(nc):
    """This container's walrus accepts at most ONE sync-wait per instruction
    on several opcodes ("Too many sync wait commands"). Hoist excess waits
    into standalone InstEventSemaphore instructions on the same engine."""
    from concourse import mybir

    cap = 1
    n = 0
    for f in nc.m.functions:
        for bb in f.blocks:
            new = []
            for inst in list(bb.instructions):
                si = inst.sync_info
                waits = list(si.on_wait) if si is not None else []
                if len(waits) > cap:
                    for j, w in enumerate(waits[cap:]):
                        new.append(
                            mybir.InstEventSemaphore(
                                name=f"{inst.name}-w{j}",
                                engine=inst.engine,
                                ins=[],
                                outs=[],
                                sync_info=mybir.SyncInfo(on_wait=[w], on_update=[]),
                            )
                        )
                        n += 1
                    inst.sync_info = mybir.SyncInfo(
                        on_wait=waits[:cap], on_update=list(si.on_update)
                    )
                new.append(inst)
            bb.instructions = new
    return n


def _build_nc():
    import concourse.bass as bass
    import concourse.tile as tile
    from concourse import mybir

    f32 = mybir.dt.float32
    f16 = mybir.dt.float16
    nc = bass.Bass(
        "TRN2",
        target_bir_lowering=False,
        debug=False,
        num_devices=NCORES,
    )
    xhi = nc.dram_tensor("xhi", [DIN, SEQ], f16, kind="ExternalInput").ap()
    xlo = nc.dram_tensor("xlo", [DIN, SEQ], f16, kind="ExternalInput").ap()
    wq = nc.dram_tensor("wq", [DIN, DO], f16, kind="ExternalInput").ap()
    wk = nc.dram_tensor("wk", [DIN, DO], f16, kind="ExternalInput").ap()
    wv = nc.dram_tensor("wv", [DIN, DO], f16, kind="ExternalInput").ap()
    mask = nc.dram_tensor("mask", [128, 128], f32, kind="ExternalInput").ap()
    outT = nc.dram_tensor("outT", [HPC, DH + 1, SEQ], f32, kind="ExternalOutput").ap()

    with tile.TileContext(nc) as tc:
        _emit_core_kernel(tc, (outT,), (xhi, xlo, wq, wk, wv, mask))
    _split_waits(nc)
    return nc


def make_mask():
    m = np.zeros((128, 128), dtype=np.float32)
    q = np.arange(128)[:, None]
    k = np.arange(128)[None, :]
    m[k > q] = NEG
    return m


def _split16(a):
    hi = a.astype(np.float16)
    lo = (a - hi.astype(np.float32)).astype(np.float16)
    return hi, lo


def shard_inputs(x, W_q, W_k, W_v):
    x = np.asarray(x, dtype=np.float32)
    W_q = np.asarray(W_q, dtype=np.float32)
    W_k = np.asarray(W_k, dtype=np.float32)
    W_v = np.asarray(W_v, dtype=np.float32)
    mask = make_mask()
    scale = 1.0 / math.sqrt(DH)
    in_maps = []
    xh_b, xl_b = [], []
    for b in range(B):
        hi, lo = _split16(np.ascontiguousarray(x[b].T))
        xh_b.append(hi)
        xl_b.append(lo)
    for c in range(NCORES):
        b, g = divmod(c, NCORES // B)
        sl = slice(g * DO, (g + 1) * DO)
        in_maps.append(
            {
                "xhi": xh_b[b],
                "xlo": xl_b[b],
                "wq": np.ascontiguousarray((W_q[:, sl] * scale).astype(np.float16)),
                "wk": np.ascontiguousarray(W_k[:, sl].astype(np.float16)),
                "wv": np.ascontiguousarray(W_v[:, sl].astype(np.float16)),
                "mask": mask,
            }
        )
    return in_maps


def assemble_output(results):
    out = np.zeros((B, SEQ, DIN), dtype=np.float32)
    for c in range(NCORES):
        b, g = divmod(c, NCORES // B)
        oT = results[c]["outT"]  # [HPC, 65, SEQ]
        for h in range(HPC):
            col = g * DO + h * DH
            out[b, :, col : col + DH] = (oT[h, :DH, :] / oT[h, DH : DH + 1, :]).T
    return out


def _install_axon_ntff_hook():
    """Provide antenv.axon_hooks (missing in this image) so trace=True works
    under axon. Mirrors trn_agent_boot.trn_boot._ntff_profile_via_ctypes."""
    import contextlib
    import ctypes
    import sys
    import types

    if "antenv.axon_hooks" in sys.modules:
        return True
    try:
        lib = ctypes.CDLL("/opt/axon/libaxon_pjrt.so")
    except OSError:
        return False
    if not hasattr(lib, "axon_start_nrt_profile"):
        return False
    lib.axon_start_nrt_profile.argtypes = [
        ctypes.POINTER(ctypes.c_int64),
        ctypes.c_size_t,
    ]
    lib.axon_start_nrt_profile.restype = ctypes.c_int64
    lib.axon_stop_nrt_profile.argtypes = [ctypes.c_char_p]
    lib.axon_stop_nrt_profile.restype = ctypes.c_int64

    @contextlib.contextmanager
    def _hook(output_dir, device_ids):
        import jax

        jax.devices()
        if device_ids:
            ids = (ctypes.c_int64 * len(device_ids))(*device_ids)
            rc = lib.axon_start_nrt_profile(ids, len(device_ids))
        else:
            rc = lib.axon_start_nrt_profile(None, 0)
        if rc != 0:
            raise RuntimeError(f"axon_start_nrt_profile rc={rc}")
        try:
            yield
        finally:
            n = lib.axon_stop_nrt_profile(str(output_dir).encode())
            print(f"ntff profile: {n} file(s) written to {output_dir}")

    mod = types.ModuleType("antenv.axon_hooks")
    holder = [_hook]
    mod.get_axon_ntff_profile_hook = lambda: holder[0]
    mod.set_axon_ntff_profile_hook = lambda h: holder.__setitem__(0, h)
    sys.modules["antenv.axon_hooks"] = mod
    import antenv

    antenv.axon_hooks = mod
    return True


def kernel(x, W_q, W_k, W_v):
    global LAST_RESULTS
    import os

    import concourse.bass_utils as bass_utils
    from concourse.bass_utils import run_bass_kernel_spmd

    if "nc" not in _CACHE:
        _CACHE["nc"] = _build_nc()
    nc = _CACHE["nc"]

    in_maps = shard_inputs(x, W_q, W_k, W_v)

    trace = bool(int(os.environ.get("MHA_TRACE", "0")))
    if trace:
        trace = _install_axon_ntff_hook()
        # avoid the fish-bucket artifact upload in this container
        bass_utils.upload_artifacts = lambda d: str(d)
    res = run_bass_kernel_spmd(
        nc, in_maps, core_ids=list(range(NCORES)), trace=trace
    )
    LAST_RESULTS = res
    return assemble_output(res.results)
